# revision 14
# baseline (speedup 1.0000x reference)
"""Trainium2 Bass kernel for CrossAttention (B=2, N=2048, C=768, H=12).

Sharding: core c -> batch b=c//4, head-group g=c%4 (3 heads each).
Each core computes Q/K/V projections for its heads over the full sequence,
attention, and a partial output projection; a 4-core ReduceScatter sums the
partials and hands each core a 512-row q-shard for residual+LayerNorm.

kernel(**inputs) takes the FULL inputs (setup_inputs() keys) and returns the
full [2, 2048, 768] output.
"""

import sys

for _p in ("/opt/trn_rl_repo",):
    if _p not in sys.path:
        sys.path.insert(0, _p)

import numpy as np

B, N, C = 2, 2048, 768
H = 12
DH = 64
EPS = 1e-5
SCALE = DH ** (-0.5)  # 0.125

NCORES = 8
GROUPS = [[0, 1, 2, 3], [4, 5, 6, 7]]
HPC = 3          # heads per core
CS = HPC * DH    # 192 output-feature slice per core
QS = N // 4      # 512 q rows per core (post reduce-scatter)
P = 128

_NC_CACHE = {}


def _build_nc():
    import concourse.bass as bass
    import concourse.mybir as mybir
    import concourse.tile as tile
    from concourse import bacc
    from concourse.masks import make_identity

    f32 = mybir.dt.float32
    Alu = mybir.AluOpType
    Act = mybir.ActivationFunctionType

    nc = bacc.Bacc(
        "TRN2",
        target_bir_lowering=False,
        debug=False,
        enable_asserts=True,
        num_devices=NCORES,
    )

    # ---- kernel I/O (per-core shapes; host shards the full problem) ----
    qT = nc.dram_tensor("qT", [C, N], f32, kind="ExternalInput").ap()
    kT = nc.dram_tensor("kT", [C, N], f32, kind="ExternalInput").ap()
    vT = nc.dram_tensor("vT", [C, N], f32, kind="ExternalInput").ap()
    wq = nc.dram_tensor("wq", [C, CS], f32, kind="ExternalInput").ap()
    wk = nc.dram_tensor("wk", [C, CS], f32, kind="ExternalInput").ap()
    wv = nc.dram_tensor("wv", [C, CS], f32, kind="ExternalInput").ap()
    wo = nc.dram_tensor("wo", [CS, C], f32, kind="ExternalInput").ap()
    bq = nc.dram_tensor("bq", [CS], f32, kind="ExternalInput").ap()
    bk = nc.dram_tensor("bk", [CS], f32, kind="ExternalInput").ap()
    bv = nc.dram_tensor("bv", [CS], f32, kind="ExternalInput").ap()
    bo = nc.dram_tensor("bo", [C], f32, kind="ExternalInput").ap()
    gamma = nc.dram_tensor("gamma", [C], f32, kind="ExternalInput").ap()
    beta = nc.dram_tensor("beta", [C], f32, kind="ExternalInput").ap()
    qres = nc.dram_tensor("qres", [QS, C], f32, kind="ExternalInput").ap()
    y = nc.dram_tensor("y", [QS, C], f32, kind="ExternalOutput").ap()

    CI = C // P          # 6 contraction chunks
    NJ = N // 512        # 4 n-chunks of 512
    NM = N // P          # 16 kv-chunks of 128
    VS = DH + 1          # 65: v columns + ones column (denominator row)
    import os
    dbg = os.environ.get("KDBG") == "1"
    if dbg:
        d_qTa = nc.dram_tensor("d_qTa", [P, N], f32, kind="ExternalOutput").ap()
        d_kTa = nc.dram_tensor("d_kTa", [P, N], f32, kind="ExternalOutput").ap()
        d_vaug = nc.dram_tensor("d_vaug", [P, 16 * 195], f32, kind="ExternalOutput").ap()
        d_o0 = nc.dram_tensor("d_o0", [DH, N], f32, kind="ExternalOutput").ap()
        d_l = nc.dram_tensor("d_l", [1, N], f32, kind="ExternalOutput").ap()
        d_rso = nc.dram_tensor("d_rso", [C, 512], f32, kind="ExternalOutput").ap()
        d_p00 = nc.dram_tensor("d_p00", [P, 1024], f32, kind="ExternalOutput").ap()
        d_opre = nc.dram_tensor("d_opre", [DH, N], f32, kind="ExternalOutput").ap()
        d_rb = nc.dram_tensor("d_rb", [DH, N], f32, kind="ExternalOutput").ap()

    with tile.TileContext(nc) as tc:
        const = tc.alloc_tile_pool(name="const", bufs=1)
        persist = tc.alloc_tile_pool(name="persist", bufs=1)
        inp = tc.alloc_tile_pool(name="inp", bufs=6)
        ppool = tc.alloc_tile_pool(name="ppool", bufs=3)
        small = tc.alloc_tile_pool(name="small", bufs=4)
        dram = tc.alloc_tile_pool(name="dram", bufs=1, space="DRAM")

        # ---- constants ----
        wq_sb = const.tile([P, CI, CS], f32, name="wq_sb")
        wk_sb = const.tile([P, CI, CS], f32, name="wk_sb")
        wv_sb = const.tile([P, CI, CS], f32, name="wv_sb")
        nc.sync.dma_start(wq_sb[:], wq.rearrange("(o p) m -> p o m", p=P))
        nc.sync.dma_start(wk_sb[:], wk.rearrange("(o p) m -> p o m", p=P))
        nc.sync.dma_start(wv_sb[:], wv.rearrange("(o p) m -> p o m", p=P))
        wo_sb = const.tile([DH, HPC, C], f32, name="wo_sb")
        nc.sync.dma_start(wo_sb[:], wo.rearrange("(h p) m -> p h m", p=DH))

        bqA = const.tile([P, 1], f32, name="bqA")
        bqB = const.tile([DH, 1], f32, name="bqB")
        bkA = const.tile([P, 1], f32, name="bkA")
        bkB = const.tile([DH, 1], f32, name="bkB")
        nc.sync.dma_start(bqA[:], bq[0:P][:, None])
        nc.sync.dma_start(bqB[:], bq[P:CS][:, None])
        nc.sync.dma_start(bkA[:], bk[0:P][:, None])
        nc.sync.dma_start(bkB[:], bk[P:CS][:, None])
        bv_b = const.tile([P, CS], f32, name="bv_b")
        bo_b = const.tile([P, C], f32, name="bo_b")
        gamma_b = const.tile([P, C], f32, name="gamma_b")
        beta_b = const.tile([P, C], f32, name="beta_b")
        nc.sync.dma_start(bv_b[0:1, :], bv[None, :])
        nc.sync.dma_start(bo_b[0:1, :], bo[None, :])
        nc.sync.dma_start(gamma_b[0:1, :], gamma[None, :])
        nc.sync.dma_start(beta_b[0:1, :], beta[None, :])
        nc.gpsimd.partition_broadcast(bv_b[:], bv_b[0:1, :])
        nc.gpsimd.partition_broadcast(bo_b[:], bo_b[0:1, :])
        nc.gpsimd.partition_broadcast(gamma_b[:], gamma_b[0:1, :])
        nc.gpsimd.partition_broadcast(beta_b[:], beta_b[0:1, :])
        ident = const.tile([P, P], f32, name="ident")
        make_identity(nc, ident)
        qres_sb = const.tile([P, QS // P, C], f32, name="qres_sb")
        nc.sync.dma_start(qres_sb[:], qres.rearrange("(t p) c -> p t c", p=P))

        # ---- persistent activations ----
        qTa = persist.tile([P, N], f32, name="qTa")    # heads 0,1 (rows 0-63 / 64-127)
        qTb = persist.tile([DH, N], f32, name="qTb")   # head 2
        kTa = persist.tile([P, N], f32, name="kTa")
        kTb = persist.tile([DH, N], f32, name="kTb")
        vaug = persist.tile([P, NM, HPC * VS], f32, name="vaug")
        # ones in the denominator column (index 64 of each 65-wide head slot)
        nc.vector.memset(
            vaug.rearrange("p m (h d) -> p m h d", d=VS)[:, :, :, DH : DH + 1], 1.0
        )
        o_h = [persist.tile([DH, N], f32, name=f"o{h}") for h in range(HPC)]
        l_sb = persist.tile([1, N], f32, name="l_sb")
        r_sb = persist.tile([1, N], f32, name="r_sb")
        rb = persist.tile([DH, N], f32, name="rb")

        rs_in = dram.tile([NJ, C, 512], f32, name="rs_in")
        rs_out = dram.tile([C, 512], f32, name="rs_out")

        # ================= Stage A: projections =================
        with tc.tile_pool(name="ppA", bufs=8, space="PSUM") as ppA:
            for j in range(NJ):
                s5 = slice(512 * j, 512 * (j + 1))
                pq_a = ppA.tile([P, 512], f32, tag="acc", name=f"pqa{j}")
                pq_b = ppA.tile([P, 512], f32, tag="acc", name=f"pqb{j}")
                pk_a = ppA.tile([P, 512], f32, tag="acc", name=f"pka{j}")
                pk_b = ppA.tile([P, 512], f32, tag="acc", name=f"pkb{j}")
                pv = [
                    ppA.tile([P, 512], f32, tag="acc", name=f"pv{j}_{m4}")
                    for m4 in range(4)
                ]
                for i in range(CI):
                    qt_t = inp.tile([P, 512], f32, tag="in", name="qt_t")
                    kt_t = inp.tile([P, 512], f32, tag="in", name="kt_t")
                    vt_t = inp.tile([P, 512], f32, tag="in", name="vt_t")
                    nc.sync.dma_start(qt_t[:], qT[P * i : P * (i + 1), s5])
                    nc.sync.dma_start(kt_t[:], kT[P * i : P * (i + 1), s5])
                    nc.sync.dma_start(vt_t[:], vT[P * i : P * (i + 1), s5])
                    st = dict(start=(i == 0), stop=(i == CI - 1))
                    nc.tensor.matmul(pq_a[:], wq_sb[:, i, 0:P], qt_t[:], **st)
                    nc.tensor.matmul(pq_b[0:DH], wq_sb[:, i, P:CS], qt_t[:], **st)
                    nc.tensor.matmul(pk_a[:], wk_sb[:, i, 0:P], kt_t[:], **st)
                    nc.tensor.matmul(pk_b[0:DH], wk_sb[:, i, P:CS], kt_t[:], **st)
                    for m4 in range(4):
                        nc.tensor.matmul(
                            pv[m4][:, 0:CS],
                            vt_t[:, P * m4 : P * (m4 + 1)],
                            wv_sb[:, i, :],
                            **st,
                        )
                # evictions (+bias)
                Alu_add = Alu.add
                nc.vector.tensor_tensor(
                    qTa[:, s5], pq_a[:], bqA.to_broadcast((P, 512)), Alu_add
                )
                nc.vector.tensor_tensor(
                    qTb[:, s5], pq_b[0:DH], bqB.to_broadcast((DH, 512)), Alu_add
                )
                nc.vector.tensor_tensor(
                    kTa[:, s5], pk_a[:], bkA.to_broadcast((P, 512)), Alu_add
                )
                nc.vector.tensor_tensor(
                    kTb[:, s5], pk_b[0:DH], bkB.to_broadcast((DH, 512)), Alu_add
                )
                for m4 in range(4):
                    m = 4 * j + m4
                    dst = vaug.rearrange("p m (h d) -> p m h d", d=VS)[
                        :, m, :, 0:DH
                    ]
                    nc.vector.tensor_tensor(
                        dst,
                        pv[m4][:, 0:CS].rearrange("p (h d) -> p h d", d=DH),
                        bv_b.rearrange("p (h d) -> p h d", d=DH),
                        Alu_add,
                    )

        # ================= Stage B: attention =================
        with (
            tc.tile_pool(name="ppS", bufs=2, space="PSUM") as ppS,
            tc.tile_pool(name="ppO", bufs=4, space="PSUM") as ppO,
        ):
            for h in range(HPC):
                if h < 2:
                    q_t = qTa[DH * h : DH * (h + 1)]
                    k_t = kTa[DH * h : DH * (h + 1)]
                else:
                    q_t = qTb[0:DH]
                    k_t = kTb[0:DH]
                po = [
                    ppO.tile([P, 512], f32, tag="o", name=f"po{h}_{qc}")
                    for qc in range(NJ)
                ]
                for m in range(NM):
                    for half in range(2):
                        ps = ppS.tile([P, 1024], f32, tag="s", name=f"ps{h}_{m}_{half}")
                        for q2 in range(2):
                            nc.tensor.matmul(
                                ps[:, 512 * q2 : 512 * (q2 + 1)],
                                k_t[:, P * m : P * (m + 1)],
                                q_t[:, 1024 * half + 512 * q2 : 1024 * half + 512 * (q2 + 1)],
                                start=True,
                                stop=True,
                            )
                        pt = ppool.tile([P, 1024], f32, tag="p", name="pt")
                        nc.scalar.activation(pt[:], ps[:], Act.Exp, scale=SCALE)
                        if dbg and h == 0 and m == 0 and half == 0:
                            nc.sync.dma_start(d_p00[:], pt[:])
                        for q2 in range(2):
                            qc = 2 * half + q2
                            nc.tensor.matmul(
                                po[qc][0:VS],
                                vaug[:, m, VS * h : VS * (h + 1)],
                                pt[:, 512 * q2 : 512 * (q2 + 1)],
                                start=(m == 0),
                                stop=(m == NM - 1),
                            )
                # evict numerator rows + denominator row, divide
                for qc in range(NJ):
                    s5 = slice(512 * qc, 512 * (qc + 1))
                    nc.vector.tensor_copy(o_h[h][:, s5], po[qc][0:DH])
                    nc.vector.tensor_copy(l_sb[0:1, s5], po[qc][DH : DH + 1])
                nc.vector.reciprocal_approx_fast(out=r_sb[:], in_=l_sb[:])
                nc.gpsimd.partition_broadcast(rb[:], r_sb[0:1, :])
                if dbg and h == 0:
                    nc.sync.dma_start(d_opre[:], o_h[h][:])
                    nc.sync.dma_start(d_rb[:], rb[:])
                nc.vector.tensor_tensor(o_h[h][:], o_h[h][:], rb[:], Alu.mult)

        if dbg:
            nc.sync.dma_start(d_qTa[:], qTa[:])
            nc.sync.dma_start(d_kTa[:], kTa[:])
            nc.sync.dma_start(d_vaug[:], vaug.rearrange("p m v -> p (m v)"))
            nc.sync.dma_start(d_o0[:], o_h[0][:])
            nc.sync.dma_start(d_l[:], l_sb[:])

        # ================= Stage C: output projection + ReduceScatter ====
        with tc.tile_pool(name="ppC", bufs=4, space="PSUM") as ppC:
            for qc in range(NJ):
                s5 = slice(512 * qc, 512 * (qc + 1))
                for co in range(CI):
                    px = ppC.tile([P, 512], f32, tag="x", name=f"px{qc}_{co}")
                    for h in range(HPC):
                        nc.tensor.matmul(
                            px[:],
                            wo_sb[:, h, P * co : P * (co + 1)],
                            o_h[h][:, s5],
                            start=(h == 0),
                            stop=(h == HPC - 1),
                        )
                    xsb = ppool.tile([P, 512], f32, tag="xsb", name="xsb")
                    nc.any.tensor_copy(xsb[:], px[:])
                    nc.sync.dma_start(rs_in[qc, P * co : P * (co + 1), :], xsb[:])

            nc.gpsimd.collective_compute(
                "ReduceScatter",
                Alu.add,
                replica_groups=GROUPS,
                ins=[rs_in.opt()],
                outs=[rs_out.opt()],
            )

        # ================= Stage D: transpose + residual + LayerNorm =====
        tD = persist.tile([P, CI, 512], f32, name="tD")
        nc.sync.dma_start(tD[:], rs_out.rearrange("(o p) w -> p o w", p=P))
        if dbg:
            nc.sync.dma_start(d_rso[:], rs_out[:])
        with tc.tile_pool(name="ppD", bufs=2, space="PSUM") as ppD:
            for qt in range(QS // P):
                pd = ppD.tile([P, C], f32, tag="d", name=f"pd{qt}")
                for co in range(CI):
                    nc.tensor.transpose(
                        pd[:, P * co : P * (co + 1)],
                        tD[:, co, P * qt : P * (qt + 1)],
                        ident[:],
                    )
                x1 = ppool.tile([P, C], f32, tag="x1", name="x1")
                nc.vector.tensor_tensor(x1[:], pd[:], qres_sb[:, qt], Alu.add)
                nc.vector.tensor_tensor(x1[:], x1[:], bo_b[:], Alu.add)
                mu = small.tile([P, 1], f32, tag="st", name="mu")
                sq = ppool.tile([P, C], f32, tag="sq", name="sq")
                sqs = small.tile([P, 1], f32, tag="st", name="sqs")
                var = small.tile([P, 1], f32, tag="st", name="var")
                rinv = small.tile([P, 1], f32, tag="st", name="rinv")
                rstd = small.tile([P, 1], f32, tag="st", name="rstd")
                nb = small.tile([P, 1], f32, tag="st", name="nb")
                nc.vector.reduce_sum(mu[:], x1[:], axis=mybir.AxisListType.X)
                nc.vector.tensor_scalar_mul(mu[:], mu[:], 1.0 / C)
                nc.vector.tensor_tensor(sq[:], x1[:], x1[:], Alu.mult)
                nc.vector.reduce_sum(sqs[:], sq[:], axis=mybir.AxisListType.X)
                nc.vector.tensor_scalar_mul(sqs[:], sqs[:], 1.0 / C)
                nc.vector.tensor_tensor(var[:], mu[:], mu[:], Alu.mult)
                nc.vector.tensor_tensor(var[:], sqs[:], var[:], Alu.subtract)
                nc.vector.tensor_scalar_add(var[:], var[:], EPS)
                nc.vector.reciprocal(rinv[:], var[:])
                nc.scalar.activation(rstd[:], rinv[:], Act.Sqrt)
                nc.vector.tensor_tensor(nb[:], mu[:], rstd[:], Alu.mult)
                nc.vector.tensor_scalar_mul(nb[:], nb[:], -1.0)
                nc.vector.tensor_scalar(
                    x1[:], x1[:], rstd[:], nb[:], Alu.mult, Alu.add
                )
                nc.vector.tensor_tensor(x1[:], x1[:], gamma_b[:], Alu.mult)
                nc.vector.tensor_tensor(x1[:], x1[:], beta_b[:], Alu.add)
                nc.sync.dma_start(
                    y.rearrange("(t p) c -> p t c", p=P)[:, qt], x1[:]
                )

        for pool in (dram, small, ppool, inp, persist, const):
            pool.release()

    nc.compile()
    return nc


def get_nc():
    if "nc" not in _NC_CACHE:
        _NC_CACHE["nc"] = _build_nc()
    return _NC_CACHE["nc"]


def make_in_maps(inputs):
    q = np.asarray(inputs["query"], np.float32)
    k = np.asarray(inputs["key_in"], np.float32)
    v = np.asarray(inputs["value"], np.float32)
    Wq = np.asarray(inputs["Wq"], np.float32)
    Wk = np.asarray(inputs["Wk"], np.float32)
    Wv = np.asarray(inputs["Wv"], np.float32)
    Wo = np.asarray(inputs["Wo"], np.float32)
    bq = np.asarray(inputs["bq"], np.float32)
    bk = np.asarray(inputs["bk"], np.float32)
    bv = np.asarray(inputs["bv"], np.float32)
    bo = np.asarray(inputs["bo"], np.float32)
    gamma = np.asarray(inputs["gamma"], np.float32)
    beta = np.asarray(inputs["beta"], np.float32)

    in_maps = []
    for c in range(NCORES):
        b, g = c // 4, c % 4
        cs = slice(CS * g, CS * (g + 1))
        in_maps.append(
            {
                "qT": np.ascontiguousarray(q[b].T),
                "kT": np.ascontiguousarray(k[b].T),
                "vT": np.ascontiguousarray(v[b].T),
                "wq": np.ascontiguousarray(Wq[:, cs]),
                "wk": np.ascontiguousarray(Wk[:, cs]),
                "wv": np.ascontiguousarray(Wv[:, cs]),
                "wo": np.ascontiguousarray(Wo[cs, :]),
                "bq": np.ascontiguousarray(bq[cs]),
                "bk": np.ascontiguousarray(bk[cs]),
                "bv": np.ascontiguousarray(bv[cs]),
                "bo": bo.copy(),
                "gamma": gamma.copy(),
                "beta": beta.copy(),
                "qres": np.ascontiguousarray(q[b, QS * g : QS * (g + 1)]),
            }
        )
    return in_maps


def _install_ntff_shim():
    """Provide antenv.axon_hooks if the image lacks it (needed for trace=True)."""
    try:
        import antenv.axon_hooks  # noqa: F401

        return
    except ImportError:
        pass
    import contextlib
    import ctypes
    import types

    so_path = "/opt/axon/libaxon_pjrt.so"
    state = {"hook": None}

    def set_axon_ntff_profile_hook(h):
        state["hook"] = h

    def get_axon_ntff_profile_hook():
        if state["hook"] is None:
            try:
                lib = ctypes.CDLL(so_path)
            except OSError:
                return None
            if not hasattr(lib, "axon_start_nrt_profile"):
                return None
            lib.axon_start_nrt_profile.argtypes = [
                ctypes.POINTER(ctypes.c_int64),
                ctypes.c_size_t,
            ]
            lib.axon_start_nrt_profile.restype = ctypes.c_int64
            lib.axon_stop_nrt_profile.argtypes = [ctypes.c_char_p]
            lib.axon_stop_nrt_profile.restype = ctypes.c_int64

            @contextlib.contextmanager
            def _hook(output_dir, device_ids):
                import jax

                jax.devices()
                if device_ids:
                    ids = (ctypes.c_int64 * len(device_ids))(*device_ids)
                    rc = lib.axon_start_nrt_profile(ids, len(device_ids))
                else:
                    rc = lib.axon_start_nrt_profile(None, 0)
                if rc != 0:
                    raise RuntimeError(f"axon_start_nrt_profile rc={rc}")
                try:
                    yield
                finally:
                    n = lib.axon_stop_nrt_profile(str(output_dir).encode())
                    print(f"profile: {n} file(s) written to {output_dir}")

            state["hook"] = _hook
        return state["hook"]

    mod = types.ModuleType("antenv.axon_hooks")
    mod.set_axon_ntff_profile_hook = set_axon_ntff_profile_hook
    mod.get_axon_ntff_profile_hook = get_axon_ntff_profile_hook
    import antenv

    antenv.axon_hooks = mod
    sys.modules["antenv.axon_hooks"] = mod


def run(inputs, trace=False, trace_cores=None):
    if trace:
        _install_ntff_shim()
    from concourse.bass_utils import run_bass_kernel_spmd

    nc = get_nc()
    in_maps = make_in_maps(inputs)
    res = run_bass_kernel_spmd(
        nc,
        in_maps,
        list(range(NCORES)),
        trace=trace,
        **({"trace_cores": trace_cores} if trace_cores is not None else {}),
    )
    out = np.empty((B, N, C), np.float32)
    for c in range(NCORES):
        b, g = c // 4, c % 4
        out[b, QS * g : QS * (g + 1)] = res.results[c]["y"]
    return out, res


def kernel(**inputs):
    out, _ = run(inputs, trace=False)
    return out


# revision 19
# speedup vs baseline: 1.1580x; 1.1580x over previous
"""Trainium2 Bass kernel for CrossAttention (B=2, N=2048, C=768, H=12).

Sharding: core c -> batch b=c//4, head-group g=c%4 (3 heads each).
Each core computes Q/K/V projections for its heads over the full sequence and
attention; an AllToAll exchanges per-head outputs so each core then computes
the full output projection, residual and LayerNorm for its own 512-row
q-shard.

kernel(**inputs) takes the FULL inputs (setup_inputs() keys) and returns the
full [2, 2048, 768] output.
"""

import sys

for _p in ("/opt/trn_rl_repo",):
    if _p not in sys.path:
        sys.path.insert(0, _p)

import numpy as np

B, N, C = 2, 2048, 768
H = 12
DH = 64
EPS = 1e-5
SCALE = DH ** (-0.5)  # 0.125

NCORES = 8
GROUPS = [[0, 1, 2, 3], [4, 5, 6, 7]]
HPC = 3          # heads per core
CS = HPC * DH    # 192 output-feature slice per core
QS = N // 4      # 512 q rows per core
P = 128

_NC_CACHE = {}


def _build_nc():
    import concourse.bass as bass
    import concourse.mybir as mybir
    import concourse.tile as tile
    from concourse import bacc

    f32 = mybir.dt.float32
    Alu = mybir.AluOpType
    Act = mybir.ActivationFunctionType

    nc = bacc.Bacc(
        "TRN2",
        target_bir_lowering=False,
        debug=False,
        enable_asserts=True,
        num_devices=NCORES,
    )

    # ---- kernel I/O (per-core shapes; host shards the full problem) ----
    qT = nc.dram_tensor("qT", [C, N], f32, kind="ExternalInput").ap()
    kT = nc.dram_tensor("kT", [C, N], f32, kind="ExternalInput").ap()
    vT = nc.dram_tensor("vT", [C, N], f32, kind="ExternalInput").ap()
    wq = nc.dram_tensor("wq", [C, CS], f32, kind="ExternalInput").ap()
    wk = nc.dram_tensor("wk", [C, CS], f32, kind="ExternalInput").ap()
    wv = nc.dram_tensor("wv", [C, CS], f32, kind="ExternalInput").ap()
    wo = nc.dram_tensor("wo", [C, C], f32, kind="ExternalInput").ap()
    bq = nc.dram_tensor("bq", [CS], f32, kind="ExternalInput").ap()
    bk = nc.dram_tensor("bk", [CS], f32, kind="ExternalInput").ap()
    bv = nc.dram_tensor("bv", [CS], f32, kind="ExternalInput").ap()
    bo = nc.dram_tensor("bo", [C], f32, kind="ExternalInput").ap()
    gamma = nc.dram_tensor("gamma", [C], f32, kind="ExternalInput").ap()
    beta = nc.dram_tensor("beta", [C], f32, kind="ExternalInput").ap()
    qres = nc.dram_tensor("qres", [QS, C], f32, kind="ExternalInput").ap()
    gsel = nc.dram_tensor("gsel", [2], f32, kind="ExternalInput").ap()
    y = nc.dram_tensor("y", [QS, C], f32, kind="ExternalOutput").ap()

    CI = C // P          # 6 contraction chunks
    NJ = N // 512        # 4 n-chunks of 512
    NM = N // P          # 16 kv-chunks of 128
    VS = DH + 1          # 65: v columns + ones column (denominator row)

    with tile.TileContext(nc) as tc:
        const = tc.alloc_tile_pool(name="const", bufs=1)
        persist = tc.alloc_tile_pool(name="persist", bufs=1)
        rows = tc.alloc_tile_pool(name="rows", bufs=2)
        ppool = tc.alloc_tile_pool(name="ppool", bufs=3)
        small = tc.alloc_tile_pool(name="small", bufs=4)
        dram = tc.alloc_tile_pool(name="dram", bufs=1, space="DRAM")

        # ---- constants ----
        wq_sb = const.tile([P, CI, CS], f32, name="wq_sb")
        wk_sb = const.tile([P, CI, CS], f32, name="wk_sb")
        wv_sb = const.tile([P, CI, CS], f32, name="wv_sb")
        nc.sync.dma_start(wq_sb[:], wq.rearrange("(o p) m -> p o m", p=P))
        nc.sync.dma_start(wk_sb[:], wk.rearrange("(o p) m -> p o m", p=P))
        nc.sync.dma_start(wv_sb[:], wv.rearrange("(o p) m -> p o m", p=P))
        wo_sb = const.tile([P, CI, C], f32, name="wo_sb")
        nc.sync.dma_start(wo_sb[:], wo.rearrange("(o p) m -> p o m", p=P))

        bqA = const.tile([P, 1], f32, name="bqA")
        bqB = const.tile([DH, 1], f32, name="bqB")
        bkA = const.tile([P, 1], f32, name="bkA")
        bkB = const.tile([DH, 1], f32, name="bkB")
        nc.sync.dma_start(bqA[:], bq[0:P][:, None])
        nc.sync.dma_start(bqB[:], bq[P:CS][:, None])
        nc.sync.dma_start(bkA[:], bk[0:P][:, None])
        nc.sync.dma_start(bkB[:], bk[P:CS][:, None])
        bv_b = const.tile([P, CS], f32, name="bv_b")
        bo_b = const.tile([P, C], f32, name="bo_b")
        gamma_b = const.tile([P, C], f32, name="gamma_b")
        beta_b = const.tile([P, C], f32, name="beta_b")
        nc.sync.dma_start(bv_b[0:1, :], bv[None, :])
        nc.sync.dma_start(bo_b[0:1, :], bo[None, :])
        nc.sync.dma_start(gamma_b[0:1, :], gamma[None, :])
        nc.sync.dma_start(beta_b[0:1, :], beta[None, :])
        nc.gpsimd.partition_broadcast(bv_b[:], bv_b[0:1, :])
        nc.gpsimd.partition_broadcast(bo_b[:], bo_b[0:1, :])
        nc.gpsimd.partition_broadcast(gamma_b[:], gamma_b[0:1, :])
        nc.gpsimd.partition_broadcast(beta_b[:], beta_b[0:1, :])
        qres_sb = const.tile([P, QS // P, C], f32, name="qres_sb")
        nc.sync.dma_start(qres_sb[:], qres.rearrange("(t p) c -> p t c", p=P))

        # ---- persistent activations ----
        qTa = persist.tile([P, N], f32, name="qTa")    # heads 0,1
        qTb = persist.tile([DH, N], f32, name="qTb")   # head 2
        kTa = persist.tile([P, N], f32, name="kTa")
        kTb = persist.tile([DH, N], f32, name="kTb")
        vaug = persist.tile([P, NM, HPC * VS], f32, name="vaug")
        nc.vector.memset(
            vaug.rearrange("p m (h d) -> p m h d", d=VS)[:, :, :, DH : DH + 1], 1.0
        )
        gs = const.tile([1, 2], f32, name="gs")
        nc.sync.dma_start(gs[:], gsel[None, :])
        s0b = const.tile([P, 1], f32, name="s0b")
        s1b = const.tile([P, 1], f32, name="s1b")
        nc.gpsimd.partition_broadcast(s0b[:], gs[0:1, 0:1])
        nc.gpsimd.partition_broadcast(s1b[:], gs[0:1, 1:2])
        o_h = [persist.tile([DH, N], f32, name=f"o{h}") for h in range(HPC)]
        l_sb = persist.tile([1, 1024], f32, name="l_sb")
        r_sb = persist.tile([1, 1024], f32, name="r_sb")
        oG = persist.tile([P, CI, QS], f32, name="oG")

        a2a_in = dram.tile([2 * NJ, CS, QS], f32, name="a2a_in")
        a2a_out = dram.tile([2 * NJ, CS, QS], f32, name="a2a_out")

        # ================= Stage A: projections =================
        with tc.tile_pool(name="ppA", bufs=8, space="PSUM") as ppA:
            # --- K ---
            pk_a = [ppA.tile([P, 512], f32, tag="acc", name=f"pka{j}") for j in range(NJ)]
            pk_b = [ppA.tile([P, 512], f32, tag="acc", name=f"pkb{j}") for j in range(NJ)]
            for i in range(CI):
                k_row = rows.tile([P, N], f32, tag="row", name="k_row")
                nc.sync.dma_start(k_row[:], kT[P * i : P * (i + 1), :])
                st = dict(start=(i == 0), stop=(i == CI - 1))
                for j in range(NJ):
                    s5 = slice(512 * j, 512 * (j + 1))
                    nc.tensor.matmul(pk_a[j][:], wk_sb[:, i, 0:P], k_row[:, s5], **st)
                    nc.tensor.matmul(pk_b[j][0:DH], wk_sb[:, i, P:CS], k_row[:, s5], **st)
            for j in range(NJ):
                s5 = slice(512 * j, 512 * (j + 1))
                nc.vector.tensor_tensor(
                    kTa[:, s5], pk_a[j][:], bkA.to_broadcast((P, 512)), Alu.add
                )
                nc.vector.tensor_tensor(
                    kTb[:, s5], pk_b[j][0:DH], bkB.to_broadcast((DH, 512)), Alu.add
                )
            # --- Q ---
            pq_a = [ppA.tile([P, 512], f32, tag="acc", name=f"pqa{j}") for j in range(NJ)]
            pq_b = [ppA.tile([P, 512], f32, tag="acc", name=f"pqb{j}") for j in range(NJ)]
            for i in range(CI):
                q_row = rows.tile([P, N], f32, tag="row", name="q_row")
                nc.sync.dma_start(q_row[:], qT[P * i : P * (i + 1), :])
                st = dict(start=(i == 0), stop=(i == CI - 1))
                for j in range(NJ):
                    s5 = slice(512 * j, 512 * (j + 1))
                    nc.tensor.matmul(pq_a[j][:], wq_sb[:, i, 0:P], q_row[:, s5], **st)
                    nc.tensor.matmul(pq_b[j][0:DH], wq_sb[:, i, P:CS], q_row[:, s5], **st)
            for j in range(NJ):
                s5 = slice(512 * j, 512 * (j + 1))
                nc.vector.tensor_tensor(
                    qTa[:, s5], pq_a[j][:], bqA.to_broadcast((P, 512)), Alu.add
                )
                nc.vector.tensor_tensor(
                    qTb[:, s5], pq_b[j][0:DH], bqB.to_broadcast((DH, 512)), Alu.add
                )
            # --- V (natural layout, accumulated per kv-chunk) ---
            for half in range(2):
                pv = [
                    ppA.tile([P, CS], f32, tag="acc", name=f"pv{half}_{m8}")
                    for m8 in range(8)
                ]
                for i in range(CI):
                    v_half = rows.tile([P, 1024], f32, tag="vrow", name="v_half")
                    nc.sync.dma_start(
                        v_half[:], vT[P * i : P * (i + 1), 1024 * half : 1024 * (half + 1)]
                    )
                    st = dict(start=(i == 0), stop=(i == CI - 1))
                    for m8 in range(8):
                        nc.tensor.matmul(
                            pv[m8][:, 0:CS],
                            v_half[:, P * m8 : P * (m8 + 1)],
                            wv_sb[:, i, :],
                            **st,
                        )
                for m8 in range(8):
                    m = 8 * half + m8
                    dst = vaug.rearrange("p m (h d) -> p m h d", d=VS)[:, m, :, 0:DH]
                    nc.vector.tensor_tensor(
                        dst,
                        pv[m8][:, 0:CS].rearrange("p (h d) -> p h d", d=DH),
                        bv_b.rearrange("p (h d) -> p h d", d=DH),
                        Alu.add,
                    )

        # ================= Stage B: attention (software-pipelined) ========
        with (
            tc.tile_pool(name="ppS", bufs=2, space="PSUM") as ppS,
            tc.tile_pool(name="ppO", bufs=4, space="PSUM") as ppO,
        ):
            for qh in range(2):
                qbase = 1024 * qh
                for h in range(HPC):
                    if h < 2:
                        q_t = qTa[DH * h : DH * (h + 1)]
                        k_t = kTa[DH * h : DH * (h + 1)]
                    else:
                        q_t = qTb[0:DH]
                        k_t = kTb[0:DH]
                    po = [
                        ppO.tile([P, 512], f32, tag="o", name=f"po{qh}_{h}_{q2}")
                        for q2 in range(2)
                    ]
                    pts = [None] * NM
                    for m in range(NM):
                        ps = ppS.tile([P, 1024], f32, tag="s", name=f"ps{qh}_{h}_{m}")
                        for q2 in range(2):
                            nc.tensor.matmul(
                                ps[:, 512 * q2 : 512 * (q2 + 1)],
                                k_t[:, P * m : P * (m + 1)],
                                q_t[:, qbase + 512 * q2 : qbase + 512 * (q2 + 1)],
                                start=True,
                                stop=True,
                            )
                        pt = ppool.tile([P, 1024], f32, tag="p", name="pt")
                        nc.scalar.activation(pt[:], ps[:], Act.Exp, scale=SCALE)
                        pts[m] = pt
                        if m >= 1:
                            for q2 in range(2):
                                nc.tensor.matmul(
                                    po[q2][0:VS],
                                    vaug[:, m - 1, VS * h : VS * (h + 1)],
                                    pts[m - 1][:, 512 * q2 : 512 * (q2 + 1)],
                                    start=(m - 1 == 0),
                                    stop=False,
                                )
                            pts[m - 1] = None
                    for q2 in range(2):
                        nc.tensor.matmul(
                            po[q2][0:VS],
                            vaug[:, NM - 1, VS * h : VS * (h + 1)],
                            pts[NM - 1][:, 512 * q2 : 512 * (q2 + 1)],
                            start=False,
                            stop=True,
                        )
                    # evict numerator + denominator, divide by softmax sum
                    for q2 in range(2):
                        s5 = slice(qbase + 512 * q2, qbase + 512 * (q2 + 1))
                        nc.vector.tensor_copy(o_h[h][:, s5], po[q2][0:DH])
                        nc.vector.tensor_copy(
                            l_sb[0:1, 512 * q2 : 512 * (q2 + 1)],
                            po[q2][DH : DH + 1],
                        )
                    sq = slice(qbase, qbase + 1024)
                    nc.vector.reciprocal_approx_fast(out=r_sb[:], in_=l_sb[:])
                    rb = ppool.tile([DH, 1024], f32, tag="rb", bufs=2, name="rb")
                    nc.gpsimd.partition_broadcast(rb[:], r_sb[0:1, :])
                    nc.vector.tensor_tensor(
                        o_h[h][:, sq], o_h[h][:, sq], rb[:], Alu.mult
                    )

        # ====== Stage C: AllToAll on per-head outputs (8 ranks; each core
        # sends its slice to both groups' slots, receivers pick their half) ==
        for r in range(NJ):
            for h in range(HPC):
                for g2 in range(2):
                    nc.sync.dma_start(
                        a2a_in[NJ * g2 + r, DH * h : DH * (h + 1), :],
                        o_h[h][:, QS * r : QS * (r + 1)],
                    )
        nc.gpsimd.collective_compute(
            "AllToAll",
            Alu.bypass,
            replica_groups=[list(range(NCORES))],
            ins=[a2a_in.opt()],
            outs=[a2a_out.opt()],
        )
        oGt = rows.tile([P, CI, QS], f32, tag="row", name="oGt")
        nc.sync.dma_start(
            oG[:],
            a2a_out[0:NJ]
            .rearrange("r s w -> (r s) w")
            .rearrange("(o p) w -> p o w", p=P),
        )
        nc.sync.dma_start(
            oGt[:],
            a2a_out[NJ : 2 * NJ]
            .rearrange("r s w -> (r s) w")
            .rearrange("(o p) w -> p o w", p=P),
        )
        nc.vector.tensor_scalar(oG[:], oG[:], s0b[:], None, Alu.mult)
        nc.vector.tensor_scalar(oGt[:], oGt[:], s1b[:], None, Alu.mult)
        nc.vector.tensor_tensor(oG[:], oG[:], oGt[:], Alu.add)

        # ======= Stage D: full Wo (token-major) + residual + LayerNorm ====
        with tc.tile_pool(name="ppD", bufs=2, space="PSUM") as ppD:
            for qt in range(QS // P):
                px = ppD.tile([P, C], f32, tag="d", name=f"px{qt}")
                for ci in range(CI):
                    st = dict(start=(ci == 0), stop=(ci == CI - 1))
                    nc.tensor.matmul(
                        px[:, 0:512],
                        oG[:, ci, P * qt : P * (qt + 1)],
                        wo_sb[:, ci, 0:512],
                        **st,
                    )
                    nc.tensor.matmul(
                        px[:, 512:C],
                        oG[:, ci, P * qt : P * (qt + 1)],
                        wo_sb[:, ci, 512:C],
                        **st,
                    )
                x1 = ppool.tile([P, C], f32, tag="x1", bufs=2, name="x1")
                nc.vector.tensor_tensor(x1[:], px[:], qres_sb[:, qt], Alu.add)
                nc.vector.tensor_tensor(x1[:], x1[:], bo_b[:], Alu.add)
                mu = small.tile([P, 1], f32, tag="st", name="mu")
                sq = ppool.tile([P, C], f32, tag="sq", bufs=2, name="sq")
                sqs = small.tile([P, 1], f32, tag="st", name="sqs")
                var = small.tile([P, 1], f32, tag="st", name="var")
                rinv = small.tile([P, 1], f32, tag="st", name="rinv")
                rstd = small.tile([P, 1], f32, tag="st", name="rstd")
                nb = small.tile([P, 1], f32, tag="st", name="nb")
                nc.vector.reduce_sum(mu[:], x1[:], axis=mybir.AxisListType.X)
                nc.vector.tensor_scalar_mul(mu[:], mu[:], 1.0 / C)
                nc.vector.tensor_tensor(sq[:], x1[:], x1[:], Alu.mult)
                nc.vector.reduce_sum(sqs[:], sq[:], axis=mybir.AxisListType.X)
                nc.vector.tensor_scalar_mul(sqs[:], sqs[:], 1.0 / C)
                nc.vector.tensor_tensor(var[:], mu[:], mu[:], Alu.mult)
                nc.vector.tensor_tensor(var[:], sqs[:], var[:], Alu.subtract)
                nc.vector.tensor_scalar_add(var[:], var[:], EPS)
                nc.vector.reciprocal(rinv[:], var[:])
                nc.scalar.activation(rstd[:], rinv[:], Act.Sqrt)
                nc.vector.tensor_tensor(nb[:], mu[:], rstd[:], Alu.mult)
                nc.vector.tensor_scalar_mul(nb[:], nb[:], -1.0)
                nc.vector.tensor_scalar(
                    x1[:], x1[:], rstd[:], nb[:], Alu.mult, Alu.add
                )
                nc.vector.tensor_tensor(x1[:], x1[:], gamma_b[:], Alu.mult)
                nc.vector.tensor_tensor(x1[:], x1[:], beta_b[:], Alu.add)
                nc.sync.dma_start(
                    y.rearrange("(t p) c -> p t c", p=P)[:, qt], x1[:]
                )

        for pool in (dram, small, ppool, rows, persist, const):
            pool.release()

    nc.compile()
    return nc


def get_nc():
    if "nc" not in _NC_CACHE:
        _NC_CACHE["nc"] = _build_nc()
    return _NC_CACHE["nc"]


def make_in_maps(inputs):
    q = np.asarray(inputs["query"], np.float32)
    k = np.asarray(inputs["key_in"], np.float32)
    v = np.asarray(inputs["value"], np.float32)
    Wq = np.asarray(inputs["Wq"], np.float32)
    Wk = np.asarray(inputs["Wk"], np.float32)
    Wv = np.asarray(inputs["Wv"], np.float32)
    Wo = np.asarray(inputs["Wo"], np.float32)
    bq = np.asarray(inputs["bq"], np.float32)
    bk = np.asarray(inputs["bk"], np.float32)
    bv = np.asarray(inputs["bv"], np.float32)
    bo = np.asarray(inputs["bo"], np.float32)
    gamma = np.asarray(inputs["gamma"], np.float32)
    beta = np.asarray(inputs["beta"], np.float32)

    in_maps = []
    for c in range(NCORES):
        b, g = c // 4, c % 4
        cs = slice(CS * g, CS * (g + 1))
        in_maps.append(
            {
                "qT": np.ascontiguousarray(q[b].T),
                "kT": np.ascontiguousarray(k[b].T),
                "vT": np.ascontiguousarray(v[b].T),
                "wq": np.ascontiguousarray(Wq[:, cs]),
                "wk": np.ascontiguousarray(Wk[:, cs]),
                "wv": np.ascontiguousarray(Wv[:, cs]),
                "wo": Wo.copy(),
                "bq": np.ascontiguousarray(bq[cs]),
                "bk": np.ascontiguousarray(bk[cs]),
                "bv": np.ascontiguousarray(bv[cs]),
                "bo": bo.copy(),
                "gamma": gamma.copy(),
                "beta": beta.copy(),
                "qres": np.ascontiguousarray(q[b, QS * g : QS * (g + 1)]),
                "gsel": np.array([1.0 - b, float(b)], np.float32),
            }
        )
    return in_maps


def _install_ntff_shim():
    """Provide antenv.axon_hooks if the image lacks it (needed for trace=True)."""
    try:
        import antenv.axon_hooks  # noqa: F401

        return
    except ImportError:
        pass
    import contextlib
    import ctypes
    import types

    so_path = "/opt/axon/libaxon_pjrt.so"
    state = {"hook": None}

    def set_axon_ntff_profile_hook(h):
        state["hook"] = h

    def get_axon_ntff_profile_hook():
        if state["hook"] is None:
            try:
                lib = ctypes.CDLL(so_path)
            except OSError:
                return None
            if not hasattr(lib, "axon_start_nrt_profile"):
                return None
            lib.axon_start_nrt_profile.argtypes = [
                ctypes.POINTER(ctypes.c_int64),
                ctypes.c_size_t,
            ]
            lib.axon_start_nrt_profile.restype = ctypes.c_int64
            lib.axon_stop_nrt_profile.argtypes = [ctypes.c_char_p]
            lib.axon_stop_nrt_profile.restype = ctypes.c_int64

            @contextlib.contextmanager
            def _hook(output_dir, device_ids):
                import jax

                jax.devices()
                if device_ids:
                    ids = (ctypes.c_int64 * len(device_ids))(*device_ids)
                    rc = lib.axon_start_nrt_profile(ids, len(device_ids))
                else:
                    rc = lib.axon_start_nrt_profile(None, 0)
                if rc != 0:
                    raise RuntimeError(f"axon_start_nrt_profile rc={rc}")
                try:
                    yield
                finally:
                    n = lib.axon_stop_nrt_profile(str(output_dir).encode())
                    print(f"profile: {n} file(s) written to {output_dir}")

            state["hook"] = _hook
        return state["hook"]

    mod = types.ModuleType("antenv.axon_hooks")
    mod.set_axon_ntff_profile_hook = set_axon_ntff_profile_hook
    mod.get_axon_ntff_profile_hook = get_axon_ntff_profile_hook
    import antenv

    antenv.axon_hooks = mod
    sys.modules["antenv.axon_hooks"] = mod


def run(inputs, trace=False, trace_cores=None):
    if trace:
        _install_ntff_shim()
    from concourse.bass_utils import run_bass_kernel_spmd

    nc = get_nc()
    in_maps = make_in_maps(inputs)
    res = run_bass_kernel_spmd(
        nc,
        in_maps,
        list(range(NCORES)),
        trace=trace,
        **({"trace_cores": trace_cores} if trace_cores is not None else {}),
    )
    out = np.empty((B, N, C), np.float32)
    for c in range(NCORES):
        b, g = c // 4, c % 4
        out[b, QS * g : QS * (g + 1)] = res.results[c]["y"]
    return out, res


def kernel(**inputs):
    out, _ = run(inputs, trace=False)
    return out


# revision 21
# speedup vs baseline: 2.1794x; 1.8820x over previous
"""Trainium2 Bass kernel for CrossAttention (B=2, N=2048, C=768, H=12).

Sharding: core c -> batch b=c//4, head-group g=c%4 (3 heads each).
Each core computes Q/K/V projections for its heads over the full sequence and
attention; an AllToAll exchanges per-head outputs so each core then computes
the full output projection, residual and LayerNorm for its own 512-row
q-shard.

kernel(**inputs) takes the FULL inputs (setup_inputs() keys) and returns the
full [2, 2048, 768] output.
"""

import sys

for _p in ("/opt/trn_rl_repo",):
    if _p not in sys.path:
        sys.path.insert(0, _p)

import numpy as np

B, N, C = 2, 2048, 768
H = 12
DH = 64
EPS = 1e-5
SCALE = DH ** (-0.5)  # 0.125

NCORES = 8
GROUPS = [[0, 1, 2, 3], [4, 5, 6, 7]]
HPC = 3          # heads per core
CS = HPC * DH    # 192 output-feature slice per core
QS = N // 4      # 512 q rows per core
P = 128

_NC_CACHE = {}


def _build_nc():
    import concourse.bass as bass
    import concourse.mybir as mybir
    import concourse.tile as tile
    from concourse import bacc

    f32 = mybir.dt.float32
    bf16 = mybir.dt.bfloat16
    Alu = mybir.AluOpType
    Act = mybir.ActivationFunctionType

    nc = bacc.Bacc(
        "TRN2",
        target_bir_lowering=False,
        debug=False,
        enable_asserts=True,
        num_devices=NCORES,
    )

    # ---- kernel I/O (per-core shapes; host shards the full problem) ----
    qT = nc.dram_tensor("qT", [C, N], bf16, kind="ExternalInput").ap()
    kT = nc.dram_tensor("kT", [C, N], bf16, kind="ExternalInput").ap()
    vT = nc.dram_tensor("vT", [C, N], bf16, kind="ExternalInput").ap()
    wq = nc.dram_tensor("wq", [C, CS], bf16, kind="ExternalInput").ap()
    wk = nc.dram_tensor("wk", [C, CS], bf16, kind="ExternalInput").ap()
    wv = nc.dram_tensor("wv", [C, CS], bf16, kind="ExternalInput").ap()
    wo = nc.dram_tensor("wo", [C, C], bf16, kind="ExternalInput").ap()
    bq = nc.dram_tensor("bq", [CS], f32, kind="ExternalInput").ap()
    bk = nc.dram_tensor("bk", [CS], f32, kind="ExternalInput").ap()
    bv = nc.dram_tensor("bv", [CS], f32, kind="ExternalInput").ap()
    bo = nc.dram_tensor("bo", [C], f32, kind="ExternalInput").ap()
    gamma = nc.dram_tensor("gamma", [C], f32, kind="ExternalInput").ap()
    beta = nc.dram_tensor("beta", [C], f32, kind="ExternalInput").ap()
    qres = nc.dram_tensor("qres", [QS, C], f32, kind="ExternalInput").ap()
    gsel = nc.dram_tensor("gsel", [2], f32, kind="ExternalInput").ap()
    y = nc.dram_tensor("y", [QS, C], f32, kind="ExternalOutput").ap()

    CI = C // P          # 6 contraction chunks
    NJ = N // 512        # 4 n-chunks of 512
    NM = N // P          # 16 kv-chunks of 128
    VS = DH + 1          # 65: v columns + ones column (denominator row)

    with tile.TileContext(nc) as tc:
        const = tc.alloc_tile_pool(name="const", bufs=1)
        persist = tc.alloc_tile_pool(name="persist", bufs=1)
        rows = tc.alloc_tile_pool(name="rows", bufs=2)
        ppool = tc.alloc_tile_pool(name="ppool", bufs=3)
        small = tc.alloc_tile_pool(name="small", bufs=4)
        dram = tc.alloc_tile_pool(name="dram", bufs=1, space="DRAM")

        # ---- constants ----
        wq_sb = const.tile([P, CI, CS], bf16, name="wq_sb")
        wk_sb = const.tile([P, CI, CS], bf16, name="wk_sb")
        wv_sb = const.tile([P, CI, CS], bf16, name="wv_sb")
        nc.sync.dma_start(wq_sb[:], wq.rearrange("(o p) m -> p o m", p=P))
        nc.sync.dma_start(wk_sb[:], wk.rearrange("(o p) m -> p o m", p=P))
        nc.sync.dma_start(wv_sb[:], wv.rearrange("(o p) m -> p o m", p=P))
        wo_sb = const.tile([P, CI, C], bf16, name="wo_sb")
        nc.sync.dma_start(wo_sb[:], wo.rearrange("(o p) m -> p o m", p=P))

        bqA = const.tile([P, 1], f32, name="bqA")
        bqB = const.tile([DH, 1], f32, name="bqB")
        bkA = const.tile([P, 1], f32, name="bkA")
        bkB = const.tile([DH, 1], f32, name="bkB")
        nc.sync.dma_start(bqA[:], bq[0:P][:, None])
        nc.sync.dma_start(bqB[:], bq[P:CS][:, None])
        nc.sync.dma_start(bkA[:], bk[0:P][:, None])
        nc.sync.dma_start(bkB[:], bk[P:CS][:, None])
        bv_b = const.tile([P, CS], f32, name="bv_b")
        bo_b = const.tile([P, C], f32, name="bo_b")
        gamma_b = const.tile([P, C], f32, name="gamma_b")
        beta_b = const.tile([P, C], f32, name="beta_b")
        nc.sync.dma_start(bv_b[0:1, :], bv[None, :])
        nc.sync.dma_start(bo_b[0:1, :], bo[None, :])
        nc.sync.dma_start(gamma_b[0:1, :], gamma[None, :])
        nc.sync.dma_start(beta_b[0:1, :], beta[None, :])
        nc.gpsimd.partition_broadcast(bv_b[:], bv_b[0:1, :])
        nc.gpsimd.partition_broadcast(bo_b[:], bo_b[0:1, :])
        nc.gpsimd.partition_broadcast(gamma_b[:], gamma_b[0:1, :])
        nc.gpsimd.partition_broadcast(beta_b[:], beta_b[0:1, :])
        qres_sb = const.tile([P, QS // P, C], f32, name="qres_sb")
        nc.sync.dma_start(qres_sb[:], qres.rearrange("(t p) c -> p t c", p=P))

        # ---- persistent activations ----
        qTa = persist.tile([P, N], bf16, name="qTa")    # heads 0,1
        qTb = persist.tile([DH, N], bf16, name="qTb")   # head 2
        kTa = persist.tile([P, N], bf16, name="kTa")
        kTb = persist.tile([DH, N], bf16, name="kTb")
        vaug = persist.tile([P, NM, HPC * VS], bf16, name="vaug")
        nc.vector.memset(
            vaug.rearrange("p m (h d) -> p m h d", d=VS)[:, :, :, DH : DH + 1], 1.0
        )
        gs = const.tile([1, 2], f32, name="gs")
        nc.sync.dma_start(gs[:], gsel[None, :])
        s0b = const.tile([P, 1], f32, name="s0b")
        s1b = const.tile([P, 1], f32, name="s1b")
        nc.gpsimd.partition_broadcast(s0b[:], gs[0:1, 0:1])
        nc.gpsimd.partition_broadcast(s1b[:], gs[0:1, 1:2])
        o_h = [persist.tile([DH, N], f32, name=f"o{h}") for h in range(HPC)]
        o_hb = [persist.tile([DH, N], bf16, name=f"ob{h}") for h in range(HPC)]
        l_sb = persist.tile([1, 1024], f32, name="l_sb")
        r_sb = persist.tile([1, 1024], f32, name="r_sb")
        oG = persist.tile([P, CI, QS], bf16, name="oG")

        a2a_in = dram.tile([2 * NJ, CS, QS], bf16, name="a2a_in")
        a2a_out = dram.tile([2 * NJ, CS, QS], bf16, name="a2a_out")

        # ================= Stage A: projections =================
        with tc.tile_pool(name="ppA", bufs=8, space="PSUM") as ppA:
            # --- K ---
            pk_a = [ppA.tile([P, 512], f32, tag="acc", name=f"pka{j}") for j in range(NJ)]
            pk_b = [ppA.tile([P, 512], f32, tag="acc", name=f"pkb{j}") for j in range(NJ)]
            for i in range(CI):
                k_row = rows.tile([P, N], bf16, tag="row", name="k_row")
                nc.sync.dma_start(k_row[:], kT[P * i : P * (i + 1), :])
                st = dict(start=(i == 0), stop=(i == CI - 1))
                for j in range(NJ):
                    s5 = slice(512 * j, 512 * (j + 1))
                    nc.tensor.matmul(pk_a[j][:], wk_sb[:, i, 0:P], k_row[:, s5], **st)
                    nc.tensor.matmul(pk_b[j][0:DH], wk_sb[:, i, P:CS], k_row[:, s5], **st)
            for j in range(NJ):
                s5 = slice(512 * j, 512 * (j + 1))
                nc.vector.tensor_tensor(
                    kTa[:, s5], pk_a[j][:], bkA.to_broadcast((P, 512)), Alu.add
                )
                nc.vector.tensor_tensor(
                    kTb[:, s5], pk_b[j][0:DH], bkB.to_broadcast((DH, 512)), Alu.add
                )
            # --- Q ---
            pq_a = [ppA.tile([P, 512], f32, tag="acc", name=f"pqa{j}") for j in range(NJ)]
            pq_b = [ppA.tile([P, 512], f32, tag="acc", name=f"pqb{j}") for j in range(NJ)]
            for i in range(CI):
                q_row = rows.tile([P, N], bf16, tag="row", name="q_row")
                nc.sync.dma_start(q_row[:], qT[P * i : P * (i + 1), :])
                st = dict(start=(i == 0), stop=(i == CI - 1))
                for j in range(NJ):
                    s5 = slice(512 * j, 512 * (j + 1))
                    nc.tensor.matmul(pq_a[j][:], wq_sb[:, i, 0:P], q_row[:, s5], **st)
                    nc.tensor.matmul(pq_b[j][0:DH], wq_sb[:, i, P:CS], q_row[:, s5], **st)
            for j in range(NJ):
                s5 = slice(512 * j, 512 * (j + 1))
                nc.vector.tensor_tensor(
                    qTa[:, s5], pq_a[j][:], bqA.to_broadcast((P, 512)), Alu.add
                )
                nc.vector.tensor_tensor(
                    qTb[:, s5], pq_b[j][0:DH], bqB.to_broadcast((DH, 512)), Alu.add
                )
            # --- V (natural layout, accumulated per kv-chunk) ---
            for half in range(2):
                pv = [
                    ppA.tile([P, CS], f32, tag="acc", name=f"pv{half}_{m8}")
                    for m8 in range(8)
                ]
                for i in range(CI):
                    v_half = rows.tile([P, 1024], bf16, tag="vrow", name="v_half")
                    nc.sync.dma_start(
                        v_half[:], vT[P * i : P * (i + 1), 1024 * half : 1024 * (half + 1)]
                    )
                    st = dict(start=(i == 0), stop=(i == CI - 1))
                    for m8 in range(8):
                        nc.tensor.matmul(
                            pv[m8][:, 0:CS],
                            v_half[:, P * m8 : P * (m8 + 1)],
                            wv_sb[:, i, :],
                            **st,
                        )
                for m8 in range(8):
                    m = 8 * half + m8
                    dst = vaug.rearrange("p m (h d) -> p m h d", d=VS)[:, m, :, 0:DH]
                    nc.vector.tensor_tensor(
                        dst,
                        pv[m8][:, 0:CS].rearrange("p (h d) -> p h d", d=DH),
                        bv_b.rearrange("p (h d) -> p h d", d=DH),
                        Alu.add,
                    )

        # ================= Stage B: attention (software-pipelined) ========
        with (
            tc.tile_pool(name="ppS", bufs=2, space="PSUM") as ppS,
            tc.tile_pool(name="ppO", bufs=4, space="PSUM") as ppO,
        ):
            for qh in range(2):
                qbase = 1024 * qh
                for h in range(HPC):
                    if h < 2:
                        q_t = qTa[DH * h : DH * (h + 1)]
                        k_t = kTa[DH * h : DH * (h + 1)]
                    else:
                        q_t = qTb[0:DH]
                        k_t = kTb[0:DH]
                    po = [
                        ppO.tile([P, 512], f32, tag="o", name=f"po{qh}_{h}_{q2}")
                        for q2 in range(2)
                    ]
                    pts = [None] * NM
                    for m in range(NM):
                        ps = ppS.tile([P, 1024], f32, tag="s", name=f"ps{qh}_{h}_{m}")
                        for q2 in range(2):
                            nc.tensor.matmul(
                                ps[:, 512 * q2 : 512 * (q2 + 1)],
                                k_t[:, P * m : P * (m + 1)],
                                q_t[:, qbase + 512 * q2 : qbase + 512 * (q2 + 1)],
                                start=True,
                                stop=True,
                            )
                        pt = ppool.tile([P, 1024], bf16, tag="p", name="pt")
                        nc.scalar.activation(pt[:], ps[:], Act.Exp, scale=SCALE)
                        pts[m] = pt
                        if m >= 1:
                            for q2 in range(2):
                                nc.tensor.matmul(
                                    po[q2][0:VS],
                                    vaug[:, m - 1, VS * h : VS * (h + 1)],
                                    pts[m - 1][:, 512 * q2 : 512 * (q2 + 1)],
                                    start=(m - 1 == 0),
                                    stop=False,
                                )
                            pts[m - 1] = None
                    for q2 in range(2):
                        nc.tensor.matmul(
                            po[q2][0:VS],
                            vaug[:, NM - 1, VS * h : VS * (h + 1)],
                            pts[NM - 1][:, 512 * q2 : 512 * (q2 + 1)],
                            start=False,
                            stop=True,
                        )
                    # evict numerator + denominator, divide by softmax sum
                    for q2 in range(2):
                        s5 = slice(qbase + 512 * q2, qbase + 512 * (q2 + 1))
                        nc.vector.tensor_copy(o_h[h][:, s5], po[q2][0:DH])
                        nc.vector.tensor_copy(
                            l_sb[0:1, 512 * q2 : 512 * (q2 + 1)],
                            po[q2][DH : DH + 1],
                        )
                    sq = slice(qbase, qbase + 1024)
                    nc.vector.reciprocal_approx_fast(out=r_sb[:], in_=l_sb[:])
                    rb = ppool.tile([DH, 1024], f32, tag="rb", bufs=2, name="rb")
                    nc.gpsimd.partition_broadcast(rb[:], r_sb[0:1, :])
                    nc.vector.tensor_tensor(
                        o_hb[h][:, sq], o_h[h][:, sq], rb[:], Alu.mult
                    )

        # ====== Stage C: AllToAll on per-head outputs (8 ranks; each core
        # sends its slice to both groups' slots, receivers pick their half) ==
        for r in range(NJ):
            for h in range(HPC):
                for g2 in range(2):
                    nc.sync.dma_start(
                        a2a_in[NJ * g2 + r, DH * h : DH * (h + 1), :],
                        o_hb[h][:, QS * r : QS * (r + 1)],
                    )
        nc.gpsimd.collective_compute(
            "AllToAll",
            Alu.bypass,
            replica_groups=[list(range(NCORES))],
            ins=[a2a_in.opt()],
            outs=[a2a_out.opt()],
        )
        oGt = rows.tile([P, CI, QS], bf16, tag="row", name="oGt")
        nc.sync.dma_start(
            oG[:],
            a2a_out[0:NJ]
            .rearrange("r s w -> (r s) w")
            .rearrange("(o p) w -> p o w", p=P),
        )
        nc.sync.dma_start(
            oGt[:],
            a2a_out[NJ : 2 * NJ]
            .rearrange("r s w -> (r s) w")
            .rearrange("(o p) w -> p o w", p=P),
        )
        nc.vector.tensor_scalar(oG[:], oG[:], s0b[:], None, Alu.mult)
        nc.vector.tensor_scalar(oGt[:], oGt[:], s1b[:], None, Alu.mult)
        nc.vector.tensor_tensor(oG[:], oG[:], oGt[:], Alu.add)

        # ======= Stage D: full Wo (token-major) + residual + LayerNorm ====
        with tc.tile_pool(name="ppD", bufs=2, space="PSUM") as ppD:
            for qt in range(QS // P):
                px = ppD.tile([P, C], f32, tag="d", name=f"px{qt}")
                for ci in range(CI):
                    st = dict(start=(ci == 0), stop=(ci == CI - 1))
                    nc.tensor.matmul(
                        px[:, 0:512],
                        oG[:, ci, P * qt : P * (qt + 1)],
                        wo_sb[:, ci, 0:512],
                        **st,
                    )
                    nc.tensor.matmul(
                        px[:, 512:C],
                        oG[:, ci, P * qt : P * (qt + 1)],
                        wo_sb[:, ci, 512:C],
                        **st,
                    )
                x1 = ppool.tile([P, C], f32, tag="x1", bufs=2, name="x1")
                nc.vector.tensor_tensor(x1[:], px[:], qres_sb[:, qt], Alu.add)
                nc.vector.tensor_tensor(x1[:], x1[:], bo_b[:], Alu.add)
                mu = small.tile([P, 1], f32, tag="st", name="mu")
                sq = ppool.tile([P, C], f32, tag="sq", bufs=2, name="sq")
                sqs = small.tile([P, 1], f32, tag="st", name="sqs")
                var = small.tile([P, 1], f32, tag="st", name="var")
                rinv = small.tile([P, 1], f32, tag="st", name="rinv")
                rstd = small.tile([P, 1], f32, tag="st", name="rstd")
                nb = small.tile([P, 1], f32, tag="st", name="nb")
                nc.vector.reduce_sum(mu[:], x1[:], axis=mybir.AxisListType.X)
                nc.vector.tensor_scalar_mul(mu[:], mu[:], 1.0 / C)
                nc.vector.tensor_tensor(sq[:], x1[:], x1[:], Alu.mult)
                nc.vector.reduce_sum(sqs[:], sq[:], axis=mybir.AxisListType.X)
                nc.vector.tensor_scalar_mul(sqs[:], sqs[:], 1.0 / C)
                nc.vector.tensor_tensor(var[:], mu[:], mu[:], Alu.mult)
                nc.vector.tensor_tensor(var[:], sqs[:], var[:], Alu.subtract)
                nc.vector.tensor_scalar_add(var[:], var[:], EPS)
                nc.vector.reciprocal(rinv[:], var[:])
                nc.scalar.activation(rstd[:], rinv[:], Act.Sqrt)
                nc.vector.tensor_tensor(nb[:], mu[:], rstd[:], Alu.mult)
                nc.vector.tensor_scalar_mul(nb[:], nb[:], -1.0)
                nc.vector.tensor_scalar(
                    x1[:], x1[:], rstd[:], nb[:], Alu.mult, Alu.add
                )
                nc.vector.tensor_tensor(x1[:], x1[:], gamma_b[:], Alu.mult)
                nc.vector.tensor_tensor(x1[:], x1[:], beta_b[:], Alu.add)
                nc.sync.dma_start(
                    y.rearrange("(t p) c -> p t c", p=P)[:, qt], x1[:]
                )

        for pool in (dram, small, ppool, rows, persist, const):
            pool.release()

    nc.compile()
    return nc


def get_nc():
    if "nc" not in _NC_CACHE:
        _NC_CACHE["nc"] = _build_nc()
    return _NC_CACHE["nc"]


def make_in_maps(inputs):
    import ml_dtypes

    b16 = ml_dtypes.bfloat16
    q = np.asarray(inputs["query"], np.float32)
    k = np.asarray(inputs["key_in"], np.float32)
    v = np.asarray(inputs["value"], np.float32)
    Wq = np.asarray(inputs["Wq"], np.float32)
    Wk = np.asarray(inputs["Wk"], np.float32)
    Wv = np.asarray(inputs["Wv"], np.float32)
    Wo = np.asarray(inputs["Wo"], np.float32)
    bq = np.asarray(inputs["bq"], np.float32)
    bk = np.asarray(inputs["bk"], np.float32)
    bv = np.asarray(inputs["bv"], np.float32)
    bo = np.asarray(inputs["bo"], np.float32)
    gamma = np.asarray(inputs["gamma"], np.float32)
    beta = np.asarray(inputs["beta"], np.float32)

    in_maps = []
    for c in range(NCORES):
        b, g = c // 4, c % 4
        cs = slice(CS * g, CS * (g + 1))
        in_maps.append(
            {
                "qT": np.ascontiguousarray(q[b].T).astype(b16),
                "kT": np.ascontiguousarray(k[b].T).astype(b16),
                "vT": np.ascontiguousarray(v[b].T).astype(b16),
                "wq": np.ascontiguousarray(Wq[:, cs]).astype(b16),
                "wk": np.ascontiguousarray(Wk[:, cs]).astype(b16),
                "wv": np.ascontiguousarray(Wv[:, cs]).astype(b16),
                "wo": Wo.astype(b16),
                "bq": np.ascontiguousarray(bq[cs]),
                "bk": np.ascontiguousarray(bk[cs]),
                "bv": np.ascontiguousarray(bv[cs]),
                "bo": bo.copy(),
                "gamma": gamma.copy(),
                "beta": beta.copy(),
                "qres": np.ascontiguousarray(q[b, QS * g : QS * (g + 1)]),
                "gsel": np.array([1.0 - b, float(b)], np.float32),
            }
        )
    return in_maps


def _install_ntff_shim():
    """Provide antenv.axon_hooks if the image lacks it (needed for trace=True)."""
    try:
        import antenv.axon_hooks  # noqa: F401

        return
    except ImportError:
        pass
    import contextlib
    import ctypes
    import types

    so_path = "/opt/axon/libaxon_pjrt.so"
    state = {"hook": None}

    def set_axon_ntff_profile_hook(h):
        state["hook"] = h

    def get_axon_ntff_profile_hook():
        if state["hook"] is None:
            try:
                lib = ctypes.CDLL(so_path)
            except OSError:
                return None
            if not hasattr(lib, "axon_start_nrt_profile"):
                return None
            lib.axon_start_nrt_profile.argtypes = [
                ctypes.POINTER(ctypes.c_int64),
                ctypes.c_size_t,
            ]
            lib.axon_start_nrt_profile.restype = ctypes.c_int64
            lib.axon_stop_nrt_profile.argtypes = [ctypes.c_char_p]
            lib.axon_stop_nrt_profile.restype = ctypes.c_int64

            @contextlib.contextmanager
            def _hook(output_dir, device_ids):
                import jax

                jax.devices()
                if device_ids:
                    ids = (ctypes.c_int64 * len(device_ids))(*device_ids)
                    rc = lib.axon_start_nrt_profile(ids, len(device_ids))
                else:
                    rc = lib.axon_start_nrt_profile(None, 0)
                if rc != 0:
                    raise RuntimeError(f"axon_start_nrt_profile rc={rc}")
                try:
                    yield
                finally:
                    n = lib.axon_stop_nrt_profile(str(output_dir).encode())
                    print(f"profile: {n} file(s) written to {output_dir}")

            state["hook"] = _hook
        return state["hook"]

    mod = types.ModuleType("antenv.axon_hooks")
    mod.set_axon_ntff_profile_hook = set_axon_ntff_profile_hook
    mod.get_axon_ntff_profile_hook = get_axon_ntff_profile_hook
    import antenv

    antenv.axon_hooks = mod
    sys.modules["antenv.axon_hooks"] = mod


def run(inputs, trace=False, trace_cores=None):
    if trace:
        _install_ntff_shim()
    from concourse.bass_utils import run_bass_kernel_spmd

    nc = get_nc()
    in_maps = make_in_maps(inputs)
    res = run_bass_kernel_spmd(
        nc,
        in_maps,
        list(range(NCORES)),
        trace=trace,
        **({"trace_cores": trace_cores} if trace_cores is not None else {}),
    )
    out = np.empty((B, N, C), np.float32)
    for c in range(NCORES):
        b, g = c // 4, c % 4
        out[b, QS * g : QS * (g + 1)] = res.results[c]["y"]
    return out, res


def kernel(**inputs):
    out, _ = run(inputs, trace=False)
    return out
